# revision 29
# baseline (speedup 1.0000x reference)
"""Trainium2 Bass kernel for nn_DLPCNNLoss (retrieval_knn).

loss = LAMDA/2 * sum(top-20 smallest same-class pairwise sq-distances per row)
       + mean(cross-entropy(x_soft, y))

Strategy:
  * Host: sort rows by class. The valid-pair mask makes the distance matrix
    block-diagonal over the 7 class blocks, cutting the GEMM ~7x.
    Core k (k<7) owns class k; core 7 is a dummy (uniform SPMD program).
  * Device (per core): resident transposed class block X^T [2048, 1248] bf16
    (scaled by sqrt(2) so the PE matmul directly yields 2*x_i.x_j).
    negd[m,n] = 2*x_m.x_n + wcomb[m] + wcomb[n] (wcomb = -sq + pad-penalty,
    symmetric) is produced in PSUM: 16 data K-chunks plus one augmentation
    K-chunk carrying {ones, wcomb}; sq is computed on-device (ACT/DVE squares
    + ones-matmul).  Only upper-triangle column blocks are matmul'd; lower
    blocks are DMA-transposed copies (negd is symmetric).
    Top-21 extraction per row via 3 rounds of vector.max + match_replace
    (rank 0 is always the self-pair, dropped); sum ranks 1..20.
    Cross-entropy per row on ACT (exp with accum, ln).
  * Host: sums per-row outputs of real rows, applies LAMDA/2 and 1/B.
"""

import numpy as np
import ml_dtypes

import concourse.bass as bass
import concourse.mybir as mybir
from concourse.tile import TileContext
from concourse.bass_utils import run_bass_kernel_spmd
from concourse.masks import make_identity

DT = mybir.dt
AF = mybir.ActivationFunctionType
ALU = mybir.AluOpType
AX = mybir.AxisListType

B, D, C = 8192, 2000, 7
LAMDA = 0.003
TOPK = 20

P = 128
DPAD = 2048          # feature dim padded to 16 K-chunks
KC = DPAD // P       # 16
NCMAX = 1280         # padded class-block width (max class size 1234 for seed 0)
TPC = 10             # M-tiles per core
NCORES = 8
PEN = 8192.0         # same-class penalty scale
CHUNKS = [(0, 512), (512, 512), (1024, NCMAX - 1024)]
SQRT2 = np.float32(np.sqrt(2.0))
BF16 = ml_dtypes.bfloat16


# --- workaround: this walrus build rejects instructions carrying more than
# one semaphore wait. Post-pass: hoist extra waits onto single-wait NOPs
# inserted immediately before the instruction (same engine, so per-engine
# program order makes the sequential waits equivalent).
def split_multi_waits(nc):
    for f in nc.m.functions:
        for b in f.blocks:
            out = []
            for ins in b.instructions:
                si = ins.sync_info
                if si is not None and si.on_wait and len(si.on_wait) > 1:
                    waits = list(si.on_wait)
                    for k, w in enumerate(waits[:-1]):
                        nop = mybir.InstNoOp(name=f"{ins.name}-sw{k}")
                        nop.engine = ins.engine
                        nop.sync_info = mybir.SyncInfo(on_wait=[w], on_update=[])
                        out.append(nop)
                    si.on_wait = waits[-1:]
                out.append(ins)
            b.instructions = out


def build_program(repeat=1):
    nc = bass.Bass()
    xin = nc.dram_tensor("xblk", [DPAD, NCMAX], DT.bfloat16, kind="ExternalInput")
    wpen_in = nc.dram_tensor("wpen", [NCMAX], DT.float32, kind="ExternalInput")
    soft_in = nc.dram_tensor("soft", [TPC, P, C], DT.float32, kind="ExternalInput")
    xsel_in = nc.dram_tensor("xsel", [TPC, P], DT.float32, kind="ExternalInput")
    lp_out = nc.dram_tensor("lp", [P, TPC], DT.float32, kind="ExternalOutput")
    ce_out = nc.dram_tensor("ce", [P, TPC], DT.float32, kind="ExternalOutput")

    with TileContext(nc) as tc:
        with (
            tc.tile_pool(name="res", bufs=1) as res,
            tc.tile_pool(name="sqs", bufs=3) as sq_pool,
            tc.tile_pool(name="small", bufs=4) as spool,
            tc.tile_pool(name="psmain", bufs=3, space="PSUM") as psmain,
            tc.tile_pool(name="pssq", bufs=1, space="PSUM") as pssq,
            tc.tile_pool(name="pstr", bufs=2, space="PSUM") as pstr,
        ):
            for _rep in range(repeat):
                _build_body(nc, res, sq_pool, spool, psmain, pssq, pstr,
                            xin, wpen_in, soft_in, xsel_in, lp_out, ce_out,
                            _rep)
    split_multi_waits(nc)
    return nc


def _build_body(nc, res, sq_pool, spool, psmain, pssq, pstr,
                xin, wpen_in, soft_in, xsel_in, lp_out, ce_out, rep):
    xa = res.tile([P, KC, NCMAX], DT.bfloat16, tag="xa", name=f"xa{rep}")
    for kc in range(KC):
        nc.sync.dma_start(xa[:, kc, :], xin[kc * P:(kc + 1) * P, :])
    wpen_sb = res.tile([1, NCMAX], DT.float32, tag="wpen", name=f"wpen{rep}")
    nc.sync.dma_start(wpen_sb[:], wpen_in[:][None, :])
    soft_sb = res.tile([P, TPC, C], DT.float32, tag="soft", name=f"soft{rep}")
    nc.sync.dma_start(soft_sb[:], soft_in[:].rearrange("t p c -> p t c"))
    xsel_sb = res.tile([P, TPC], DT.float32, tag="xsel", name=f"xsel{rep}")
    nc.sync.dma_start(xsel_sb[:], xsel_in[:].rearrange("t p -> p t"))

    halves = res.tile([P, 1], DT.bfloat16, tag="halves", name=f"halves{rep}")
    nc.vector.memset(halves[:], 0.5)
    ident = res.tile([P, P], DT.bfloat16, tag="ident", name=f"ident{rep}")
    make_identity(nc, ident[:])

    # ---- sq over block columns: sq[n] = 0.5 * sum_d (sqrt2*x_n)_d^2 ----
    # squares split across ACT and GPSIMD so sq is ready sooner
    sq_ps = pssq.tile([1, NCMAX], DT.float32, tag="sqps", name=f"sqps{rep}")
    for kc in range(KC):
        sc = sq_pool.tile([P, NCMAX], DT.bfloat16, tag="sqscratch")
        if kc % 2 == 0:
            nc.scalar.activation(sc[:], xa[:, kc, :], AF.Square)
        else:
            nc.vector.tensor_tensor(sc[:], xa[:, kc, :], xa[:, kc, :], ALU.mult)
        for (o, w) in CHUNKS:
            nc.tensor.matmul(
                sq_ps[:, o:o + w], halves[:], sc[:, o:o + w],
                start=(kc == 0), stop=(kc == KC - 1),
            )
    sqf = spool.tile([1, NCMAX], DT.float32, tag="sqf", name=f"sqf{rep}")
    for (o, w) in CHUNKS:
        nc.scalar.activation(sqf[:, o:o + w], sq_ps[:, o:o + w], AF.Copy)
    # w_comb[n] = wpen[n] - sq[n]; applied on both axes (negd symmetric)
    wcomb_bf = spool.tile([1, NCMAX], DT.bfloat16, tag="wcombbf", name=f"wcombbf{rep}")
    nc.gpsimd.tensor_tensor(wcomb_bf[:], wpen_sb[:], sqf[:], ALU.subtract)
    # augmentation K-chunk: negd_aug[m,n] = 1*wcomb[n] + wcomb[m]*1
    ones_row = spool.tile([1, NCMAX], DT.bfloat16, tag="onesrow", name=f"ones{rep}")
    nc.gpsimd.memset(ones_row[:], 1.0)
    aug_v = res.tile([P, NCMAX], DT.bfloat16, tag="augv", name=f"augv{rep}")
    nc.gpsimd.memset(aug_v[:], 0.0)
    nc.sync.dma_start(aug_v[0:1, :], ones_row[:])
    nc.sync.dma_start(aug_v[1:2, :], wcomb_bf[:])
    aug_w = res.tile([P, NCMAX], DT.bfloat16, tag="augw", name=f"augw{rep}")
    nc.gpsimd.memset(aug_w[:], 0.0)
    nc.sync.dma_start(aug_w[0:1, :], wcomb_bf[:])
    nc.sync.dma_start(aug_w[1:2, :], ones_row[:])

    lp_sb = res.tile([P, TPC], DT.float32, tag="lpsb", name=f"lpsb{rep}")
    nc.vector.memset(lp_sb[:], 0.0)
    ce_sb = res.tile([P, TPC], DT.float32, tag="cesb", name=f"cesb{rep}")
    nc.vector.memset(ce_sb[:], 0.0)

    # cross-entropy for all rows first: independent of the distance pipeline,
    # runs while the block DMA / sq gate is still filling
    for t in range(TPC):
        mP = min(P, NCMAX - t * P)
        st = soft_sb[:mP, t, :]
        mx = spool.tile([P, 1], DT.float32, tag="mx")
        nc.vector.tensor_reduce(mx[:mP], st, axis=AX.X, op=ALU.max)
        nmx = spool.tile([P, 1], DT.float32, tag="nmx")
        nc.vector.tensor_scalar_mul(nmx[:mP], mx[:mP], -1.0)
        ex = spool.tile([P, C], DT.float32, tag="ex")
        se = spool.tile([P, 1], DT.float32, tag="se")
        nc.scalar.activation(ex[:mP], st, AF.Exp,
                             bias=nmx[:mP], accum_out=se[:mP])
        ln = spool.tile([P, 1], DT.float32, tag="ln")
        nc.scalar.activation(ln[:mP], se[:mP], AF.Ln)
        tmp = spool.tile([P, 1], DT.float32, tag="tmp")
        nc.vector.tensor_add(tmp[:mP], ln[:mP], mx[:mP])
        nc.vector.tensor_sub(ce_sb[:mP, t:t + 1], tmp[:mP], xsel_sb[:mP, t:t + 1])
    nc.sync.dma_start(ce_out[:], ce_sb[:])

    # all tiles' negd rows resident: transposed blocks land across tiles
    negd_all = res.tile([P, TPC, NCMAX], DT.bfloat16, tag="negd_all",
                        name=f"negdall{rep}")
    for t in range(TPC):
        m0 = t * P
        mP = min(P, NCMAX - m0)  # 128, or 96 for the last tile
        negd = negd_all[:, t, :]
        # matmul only the upper-triangle column range [m0, NCMAX)
        o = m0
        while o < NCMAX:
            w = min(512, NCMAX - o)
            ps = psmain.tile([P, 512], DT.float32, tag="psmain",
                             name=f"ps{rep}_{t}_{o}")
            for kc in range(KC):
                nc.tensor.matmul(
                    ps[:mP, :w],
                    xa[:, kc, m0:m0 + mP],
                    xa[:, kc, o:o + w],
                    start=(kc == 0), stop=False,
                )
            nc.tensor.matmul(
                ps[:mP, :w], aug_v[:, m0:m0 + mP], aug_w[:, o:o + w],
                start=False, stop=True,
            )
            nc.scalar.activation(negd[:mP, o:o + w], ps[:mP, :w], AF.Copy)
            o += w
        # scatter transposes into later tiles' rows (negd symmetric)
        for u in range(t + 1, TPC):
            u0 = u * P
            wu = min(P, NCMAX - u0)
            ptr = pstr.tile([P, P], DT.bfloat16, tag="pstr",
                            name=f"ptr{rep}_{t}_{u}")
            nc.tensor.transpose(
                ptr[:wu, :mP], negd_all[:mP, t, u0:u0 + wu], ident[:mP, :mP])
            nc.scalar.activation(
                negd_all[:wu, u, m0:m0 + mP], ptr[:wu, :mP], AF.Copy)
        # round-1 max reads the resident row directly (read-only) while GPSIMD
        # snapshots it; later rounds zap the scratch copy, so the resident row
        # (still needed as a transpose source) stays intact
        cand = spool.tile([P, 24], DT.bfloat16, tag="cand")
        nc.vector.max(out=cand[:mP, 0:8], in_=negd[:mP])
        exsc = sq_pool.tile([P, NCMAX], DT.bfloat16, tag="exsc")
        nc.gpsimd.tensor_copy(exsc[:mP], negd[:mP])
        nc.vector.match_replace(
            out=exsc[:mP], in_to_replace=cand[:mP, 0:8],
            in_values=exsc[:mP], imm_value=-3e38)
        nc.vector.max(out=cand[:mP, 8:16], in_=exsc[:mP])
        nc.vector.match_replace(
            out=exsc[:mP], in_to_replace=cand[:mP, 8:16],
            in_values=exsc[:mP], imm_value=-3e38)
        nc.vector.max(out=cand[:mP, 16:24], in_=exsc[:mP])
        nc.vector.tensor_reduce(
            lp_sb[:mP, t:t + 1], cand[:mP, 1:21], axis=AX.X, op=ALU.add)

    nc.sync.dma_start(lp_out[:], lp_sb[:])


_program_cache = {}


def get_program():
    if "nc" not in _program_cache:
        _program_cache["nc"] = build_program()
    return _program_cache["nc"]


def build_core_inputs(x_soft, x_feat, y):
    """Host-side sharding: per-core input dicts + masks for recombination."""
    x_soft = np.ascontiguousarray(np.asarray(x_soft, dtype=np.float32))
    x_feat = np.ascontiguousarray(np.asarray(x_feat, dtype=np.float32))
    y = np.asarray(y).astype(np.int64)

    perm = np.argsort(y, kind="stable")
    ys = y[perm]
    sizes = np.bincount(ys, minlength=C)
    assert sizes.max() <= NCMAX, f"class too big for NCMAX: {sizes}"
    assert (sizes >= TOPK + 1).all(), f"class too small: {sizes}"
    starts = np.concatenate([[0], np.cumsum(sizes)])

    scaled = (x_feat * SQRT2).astype(BF16)

    in_maps = []
    n_real = []
    for k in range(NCORES):
        xblk = np.zeros((DPAD, NCMAX), dtype=BF16)
        soft = np.zeros((TPC, P, C), dtype=np.float32)
        xsel = np.zeros((TPC, P), dtype=np.float32)
        wpen = np.full(NCMAX, -PEN * 99.0 ** 2, dtype=np.float32)
        if k < C:
            n_c = int(sizes[k])
            rows = perm[starts[k]:starts[k + 1]]
            xblk[:D, :n_c] = scaled[rows].T
            wpen[:n_c] = 0.0
            sf = x_soft[rows]
            soft.reshape(TPC * P, C)[:n_c] = sf
            xsel.reshape(TPC * P)[:n_c] = sf[np.arange(n_c), y[rows]]
            n_real.append(n_c)
        else:
            n_real.append(0)
        in_maps.append({
            "xblk": xblk, "wpen": wpen,
            "soft": soft, "xsel": xsel,
        })
    return in_maps, n_real


def combine_outputs(results, n_real):
    col = np.arange(TPC)[None, :] * P + np.arange(P)[:, None]  # [P, TPC]
    lp_sum = 0.0
    ce_sum = 0.0
    for k in range(NCORES):
        if n_real[k] == 0:
            continue
        mask = col < n_real[k]
        lp_sum += float(results[k]["lp"][mask].sum(dtype=np.float64))
        ce_sum += float(results[k]["ce"][mask].sum(dtype=np.float64))
    loss_lp = -lp_sum
    return np.asarray(LAMDA * loss_lp / 2.0 + ce_sum / B, dtype=np.float32)


def run(x_soft, x_feat, y, **spmd_kwargs):
    nc = get_program()
    in_maps, n_real = build_core_inputs(x_soft, x_feat, y)
    res = run_bass_kernel_spmd(nc, in_maps, core_ids=list(range(NCORES)), **spmd_kwargs)
    return combine_outputs(res.results, n_real), res


def kernel(x_soft, x_feat, y):
    out, _ = run(x_soft, x_feat, y)
    return out


# revision 35
# speedup vs baseline: 1.0444x; 1.0444x over previous
"""Trainium2 Bass kernel for nn_DLPCNNLoss (retrieval_knn).

loss = LAMDA/2 * sum(top-20 smallest same-class pairwise sq-distances per row)
       + mean(cross-entropy(x_soft, y))

Strategy:
  * Host: sort rows by class. The valid-pair mask makes the distance matrix
    block-diagonal over the 7 class blocks, cutting the GEMM ~7x.
    Core k (k<7) owns class k; core 7 is a dummy (uniform SPMD program).
  * Device (per core): resident transposed class block X^T [2048, 1248] bf16
    (scaled by sqrt(2) so the PE matmul directly yields 2*x_i.x_j).
    negd[m,n] = 2*x_m.x_n + wcomb[m] + wcomb[n] (wcomb = -sq + pad-penalty,
    symmetric) is produced in PSUM: 16 data K-chunks plus one augmentation
    K-chunk carrying {ones, wcomb}; sq is computed on-device (ACT/DVE squares
    + ones-matmul).  Only upper-triangle column blocks are matmul'd; lower
    blocks are PE-transposed copies (negd is symmetric; NB: SBUF-to-SBUF DMA
    transpose hangs this device on NEFF re-execution, so PE is used).
    Top-21 extraction per row via 3 rounds of vector.max + match_replace
    (rank 0 is always the self-pair, dropped); sum ranks 1..20.
    Cross-entropy per row on ACT (exp with accum, ln).
  * Host: sums per-row outputs of real rows, applies LAMDA/2 and 1/B.
"""

import numpy as np
import ml_dtypes

import concourse.bass as bass
import concourse.mybir as mybir
from concourse.tile import TileContext
from concourse.bass_utils import run_bass_kernel_spmd
from concourse.masks import make_identity

DT = mybir.dt
AF = mybir.ActivationFunctionType
ALU = mybir.AluOpType
AX = mybir.AxisListType

B, D, C = 8192, 2000, 7
LAMDA = 0.003
TOPK = 20

P = 128
DPAD = 2048          # feature dim padded to 16 K-chunks
KC = DPAD // P       # 16
NCMAX = 1248         # padded class-block width (max class size 1234 for seed 0)
TPC = 10             # M-tiles per core
NCORES = 8
PEN = 8192.0         # same-class penalty scale
CHUNKS = [(0, 512), (512, 512), (1024, NCMAX - 1024)]
SQRT2 = np.float32(np.sqrt(2.0))
BF16 = ml_dtypes.bfloat16


# --- workaround: this walrus build rejects instructions carrying more than
# one semaphore wait. Post-pass: hoist extra waits onto single-wait NOPs
# inserted immediately before the instruction (same engine, so per-engine
# program order makes the sequential waits equivalent).
def split_multi_waits(nc):
    for f in nc.m.functions:
        for b in f.blocks:
            out = []
            for ins in b.instructions:
                si = ins.sync_info
                if si is not None and si.on_wait and len(si.on_wait) > 1:
                    waits = list(si.on_wait)
                    for k, w in enumerate(waits[:-1]):
                        nop = mybir.InstNoOp(name=f"{ins.name}-sw{k}")
                        nop.engine = ins.engine
                        nop.sync_info = mybir.SyncInfo(on_wait=[w], on_update=[])
                        out.append(nop)
                    si.on_wait = waits[-1:]
                out.append(ins)
            b.instructions = out


def build_program(repeat=1):
    nc = bass.Bass()
    xin = nc.dram_tensor("xblk", [DPAD, NCMAX], DT.bfloat16, kind="ExternalInput")
    wpen_in = nc.dram_tensor("wpen", [NCMAX], DT.float32, kind="ExternalInput")
    soft_in = nc.dram_tensor("soft", [TPC, P, C], DT.float32, kind="ExternalInput")
    xsel_in = nc.dram_tensor("xsel", [TPC, P], DT.float32, kind="ExternalInput")
    lp_out = nc.dram_tensor("lp", [P, TPC], DT.float32, kind="ExternalOutput")
    ce_out = nc.dram_tensor("ce", [P, TPC], DT.float32, kind="ExternalOutput")

    with TileContext(nc) as tc:
        with (
            tc.tile_pool(name="res", bufs=1) as res,
            tc.tile_pool(name="sqs", bufs=3) as sq_pool,
            tc.tile_pool(name="small", bufs=4) as spool,
            tc.tile_pool(name="psmain", bufs=4, space="PSUM") as psmain,
            tc.tile_pool(name="pssq", bufs=1, space="PSUM") as pssq,
            tc.tile_pool(name="pstr", bufs=1, space="PSUM") as pstr,
        ):
            for _rep in range(repeat):
                _build_body(nc, res, sq_pool, spool, psmain, pssq, pstr,
                            xin, wpen_in, soft_in, xsel_in, lp_out, ce_out,
                            _rep)
    split_multi_waits(nc)
    return nc


def _build_body(nc, res, sq_pool, spool, psmain, pssq, pstr,
                xin, wpen_in, soft_in, xsel_in, lp_out, ce_out, rep):
    xa = res.tile([P, KC, NCMAX], DT.bfloat16, tag="xa", name=f"xa{rep}")
    for kc in range(KC):
        nc.sync.dma_start(xa[:, kc, :], xin[kc * P:(kc + 1) * P, :])
    wpen_sb = res.tile([1, NCMAX], DT.float32, tag="wpen", name=f"wpen{rep}")
    nc.sync.dma_start(wpen_sb[:], wpen_in[:][None, :])
    soft_sb = res.tile([P, TPC, C], DT.float32, tag="soft", name=f"soft{rep}")
    nc.sync.dma_start(soft_sb[:], soft_in[:].rearrange("t p c -> p t c"))
    xsel_sb = res.tile([P, TPC], DT.float32, tag="xsel", name=f"xsel{rep}")
    nc.sync.dma_start(xsel_sb[:], xsel_in[:].rearrange("t p -> p t"))

    halves = res.tile([P, 1], DT.bfloat16, tag="halves", name=f"halves{rep}")
    nc.vector.memset(halves[:], 0.5)
    ident = res.tile([P, P], DT.bfloat16, tag="ident", name=f"ident{rep}")
    make_identity(nc, ident[:])

    # ---- sq over block columns: sq[n] = 0.5 * sum_d (sqrt2*x_n)_d^2 ----
    # squares split across ACT and GPSIMD so sq is ready sooner
    sq_ps = pssq.tile([1, NCMAX], DT.float32, tag="sqps", name=f"sqps{rep}")
    for kc in range(KC):
        sc = sq_pool.tile([P, NCMAX], DT.bfloat16, tag="sqscratch")
        if kc % 2 == 0:
            nc.scalar.activation(sc[:], xa[:, kc, :], AF.Square)
        else:
            nc.vector.tensor_tensor(sc[:], xa[:, kc, :], xa[:, kc, :], ALU.mult)
        for (o, w) in CHUNKS:
            nc.tensor.matmul(
                sq_ps[:, o:o + w], halves[:], sc[:, o:o + w],
                start=(kc == 0), stop=(kc == KC - 1),
            )
    # w_comb[n] = wpen[n] - sq[n]; applied on both axes (negd symmetric).
    # Filled chunk-by-chunk so early chains' aug matmuls fire as soon as the
    # chunk's sq lands rather than after the whole row.
    sqf = spool.tile([1, NCMAX], DT.float32, tag="sqf", name=f"sqf{rep}")
    wcomb_bf = spool.tile([1, NCMAX], DT.bfloat16, tag="wcombbf", name=f"wcombbf{rep}")
    ones_row = spool.tile([1, NCMAX], DT.bfloat16, tag="onesrow", name=f"ones{rep}")
    nc.gpsimd.memset(ones_row[:], 1.0)
    aug_v = res.tile([P, NCMAX], DT.bfloat16, tag="augv", name=f"augv{rep}")
    nc.gpsimd.memset(aug_v[:], 0.0)
    nc.sync.dma_start(aug_v[0:1, :], ones_row[:])
    aug_w = res.tile([P, NCMAX], DT.bfloat16, tag="augw", name=f"augw{rep}")
    nc.gpsimd.memset(aug_w[:], 0.0)
    nc.sync.dma_start(aug_w[1:2, :], ones_row[:])
    for (o, w) in CHUNKS:
        nc.scalar.activation(sqf[:, o:o + w], sq_ps[:, o:o + w], AF.Copy)
        nc.gpsimd.tensor_tensor(wcomb_bf[:, o:o + w], wpen_sb[:, o:o + w],
                                sqf[:, o:o + w], ALU.subtract)
        nc.sync.dma_start(aug_v[1:2, o:o + w], wcomb_bf[:, o:o + w])
        nc.sync.dma_start(aug_w[0:1, o:o + w], wcomb_bf[:, o:o + w])

    lp_sb = res.tile([P, TPC], DT.float32, tag="lpsb", name=f"lpsb{rep}")
    nc.vector.memset(lp_sb[:], 0.0)
    ce_sb = res.tile([P, TPC], DT.float32, tag="cesb", name=f"cesb{rep}")
    nc.vector.memset(ce_sb[:], 0.0)

    # cross-entropy for all rows first: independent of the distance pipeline,
    # runs while the block DMA / sq gate is still filling
    for t in range(TPC):
        mP = min(P, NCMAX - t * P)
        st = soft_sb[:mP, t, :]
        mx = spool.tile([P, 1], DT.float32, tag="mx")
        nc.vector.tensor_reduce(mx[:mP], st, axis=AX.X, op=ALU.max)
        nmx = spool.tile([P, 1], DT.float32, tag="nmx")
        nc.vector.tensor_scalar_mul(nmx[:mP], mx[:mP], -1.0)
        ex = spool.tile([P, C], DT.float32, tag="ex")
        se = spool.tile([P, 1], DT.float32, tag="se")
        nc.scalar.activation(ex[:mP], st, AF.Exp,
                             bias=nmx[:mP], accum_out=se[:mP])
        ln = spool.tile([P, 1], DT.float32, tag="ln")
        nc.scalar.activation(ln[:mP], se[:mP], AF.Ln)
        tmp = spool.tile([P, 1], DT.float32, tag="tmp")
        nc.vector.tensor_add(tmp[:mP], ln[:mP], mx[:mP])
        nc.vector.tensor_sub(ce_sb[:mP, t:t + 1], tmp[:mP], xsel_sb[:mP, t:t + 1])
    nc.sync.dma_start(ce_out[:], ce_sb[:])

    # all tiles' negd rows resident: transposed blocks land across tiles
    negd_all = res.tile([P, TPC, NCMAX], DT.bfloat16, tag="negd_all",
                        name=f"negdall{rep}")
    for t in range(TPC):
        m0 = t * P
        mP = min(P, NCMAX - m0)  # 128, or 96 for the last tile
        negd = negd_all[:, t, :]
        # matmul only the upper-triangle column range [m0, NCMAX)
        o = m0
        while o < NCMAX:
            w = min(512, NCMAX - o)
            ps = psmain.tile([P, 512], DT.float32, tag="psmain",
                             name=f"ps{rep}_{t}_{o}")
            for kc in range(KC):
                nc.tensor.matmul(
                    ps[:mP, :w],
                    xa[:, kc, m0:m0 + mP],
                    xa[:, kc, o:o + w],
                    start=(kc == 0), stop=False,
                )
            nc.tensor.matmul(
                ps[:mP, :w], aug_v[:, m0:m0 + mP], aug_w[:, o:o + w],
                start=False, stop=True,
            )
            nc.scalar.activation(negd[:mP, o:o + w], ps[:mP, :w], AF.Copy)
            o += w
        # scatter transposes into later tiles' rows (negd symmetric)
        for u in range(t + 1, TPC):
            u0 = u * P
            wu = min(P, NCMAX - u0)
            ptr = pstr.tile([P, P], DT.bfloat16, tag="pstr",
                            name=f"ptr{rep}_{t}_{u}")
            nc.tensor.transpose(
                ptr[:wu, :mP], negd_all[:mP, t, u0:u0 + wu], ident[:mP, :mP])
            nc.scalar.activation(
                negd_all[:wu, u, m0:m0 + mP], ptr[:wu, :mP], AF.Copy)
        # round-1 max reads the resident row directly (read-only) while GPSIMD
        # snapshots it; later rounds zap the scratch copy, so the resident row
        # (still needed as a transpose source) stays intact
        cand = spool.tile([P, 24], DT.bfloat16, tag="cand")
        nc.vector.max(out=cand[:mP, 0:8], in_=negd[:mP])
        exsc = sq_pool.tile([P, NCMAX], DT.bfloat16, tag="exsc")
        nc.gpsimd.tensor_copy(exsc[:mP], negd[:mP])
        nc.vector.match_replace(
            out=exsc[:mP], in_to_replace=cand[:mP, 0:8],
            in_values=exsc[:mP], imm_value=-3e38)
        nc.vector.max(out=cand[:mP, 8:16], in_=exsc[:mP])
        nc.vector.match_replace(
            out=exsc[:mP], in_to_replace=cand[:mP, 8:16],
            in_values=exsc[:mP], imm_value=-3e38)
        nc.vector.max(out=cand[:mP, 16:24], in_=exsc[:mP])
        nc.vector.tensor_reduce(
            lp_sb[:mP, t:t + 1], cand[:mP, 1:21], axis=AX.X, op=ALU.add)

    nc.sync.dma_start(lp_out[:], lp_sb[:])


_program_cache = {}


def get_program():
    if "nc" not in _program_cache:
        _program_cache["nc"] = build_program()
    return _program_cache["nc"]


def build_core_inputs(x_soft, x_feat, y):
    """Host-side sharding: per-core input dicts + masks for recombination."""
    x_soft = np.ascontiguousarray(np.asarray(x_soft, dtype=np.float32))
    x_feat = np.ascontiguousarray(np.asarray(x_feat, dtype=np.float32))
    y = np.asarray(y).astype(np.int64)

    perm = np.argsort(y, kind="stable")
    ys = y[perm]
    sizes = np.bincount(ys, minlength=C)
    assert sizes.max() <= NCMAX, f"class too big for NCMAX: {sizes}"
    assert (sizes >= TOPK + 1).all(), f"class too small: {sizes}"
    starts = np.concatenate([[0], np.cumsum(sizes)])

    scaled = (x_feat * SQRT2).astype(BF16)

    in_maps = []
    n_real = []
    for k in range(NCORES):
        xblk = np.zeros((DPAD, NCMAX), dtype=BF16)
        soft = np.zeros((TPC, P, C), dtype=np.float32)
        xsel = np.zeros((TPC, P), dtype=np.float32)
        wpen = np.full(NCMAX, -PEN * 99.0 ** 2, dtype=np.float32)
        if k < C:
            n_c = int(sizes[k])
            rows = perm[starts[k]:starts[k + 1]]
            xblk[:D, :n_c] = scaled[rows].T
            wpen[:n_c] = 0.0
            sf = x_soft[rows]
            soft.reshape(TPC * P, C)[:n_c] = sf
            xsel.reshape(TPC * P)[:n_c] = sf[np.arange(n_c), y[rows]]
            n_real.append(n_c)
        else:
            n_real.append(0)
        in_maps.append({
            "xblk": xblk, "wpen": wpen,
            "soft": soft, "xsel": xsel,
        })
    return in_maps, n_real


def combine_outputs(results, n_real):
    col = np.arange(TPC)[None, :] * P + np.arange(P)[:, None]  # [P, TPC]
    lp_sum = 0.0
    ce_sum = 0.0
    for k in range(NCORES):
        if n_real[k] == 0:
            continue
        mask = col < n_real[k]
        lp_sum += float(results[k]["lp"][mask].sum(dtype=np.float64))
        ce_sum += float(results[k]["ce"][mask].sum(dtype=np.float64))
    loss_lp = -lp_sum
    return np.asarray(LAMDA * loss_lp / 2.0 + ce_sum / B, dtype=np.float32)


def run(x_soft, x_feat, y, **spmd_kwargs):
    nc = get_program()
    in_maps, n_real = build_core_inputs(x_soft, x_feat, y)
    res = run_bass_kernel_spmd(nc, in_maps, core_ids=list(range(NCORES)), **spmd_kwargs)
    return combine_outputs(res.results, n_real), res


def kernel(x_soft, x_feat, y):
    out, _ = run(x_soft, x_feat, y)
    return out
